# revision 34
# baseline (speedup 1.0000x reference)
"""MoE FFN (EnterpriseFFN) Trainium2 kernel.

8192 tokens x d_model=1024, 8 experts (hidden 512), top-2 gating where every
selected expert is scaled by the SUM of the top-2 softmax gates.

Distribution: data-parallel over tokens -- each of the 8 NeuronCores runs
1024 tokens through all 8 experts (dense compute, masked combine, exactly
like the reference einsum formulation). Expert weights are replicated.

Per-core pipeline (activations kept transposed, [feature, token]):
  1. Load x [1024 tok, 1024 d]; PE-transpose to fp32 xg (gating) and bf16 xT
     (FFN) tiles, with per-chunk gating (softmax + top-2 via max / masked-max
     on DVE, exact fp32 logits so the top-2 selection matches the oracle);
     S[tok, e] = sel * tok_w is PE-transposed to ST [e, tok]. Expert 0's
     layer 1 is interleaved to keep the PE stream dense (HAM warm).
  2. Per expert e: hT = gelu(w1[e].T-chunks @ xT + b1) on PE/ACT (bf16 in,
     fp32 PSUM), scaled along tokens by a ones-matmul broadcast of ST's row;
     layer 2 accumulates expert PAIRS plus the rank-8 b2 @ S matmul in PSUM;
     a fp32 SBUF accumulator sums the pairs.
  3. Store yT [d, tok]; the host transposes shards back and concatenates.

FFN matmuls run in bf16 (fast weight load, 1 cyc/row); gating runs in exact
fp32. Weight tiles are DMA-staged fp32 then cast to bf16 on ACT/DVE.
"""

import numpy as np

import bass_rust
import concourse.bass as bass
import concourse.tile as tile
from concourse import mybir
from concourse.bass_utils import run_bass_kernel_spmd
from concourse.masks import make_identity
from concourse.tile_rust import add_dep_helper

N_CORES = 8
B, S, D, H, E = 4, 2048, 1024, 512, 8
NTOK = B * S          # 8192 total tokens
TOK = NTOK // N_CORES  # 1024 tokens per core
KD = D // 128          # 8 d_model chunks
KH = H // 128          # 4 hidden chunks
TT = TOK // 128        # 8 token chunks
NF = 512               # matmul moving free width
NHF = TOK // NF        # 2 token halves

FP = mybir.dt.float32
BF = mybir.dt.bfloat16
AF = mybir.ActivationFunctionType
ALU = mybir.AluOpType
AX = mybir.AxisListType


def _legalize_sync_waits(nc, max_waits=1):
    """Split multi-wait instructions for this walrus (1 sync wait per inst).

    Any instruction carrying more than ``max_waits`` sync-wait commands gets
    the extra waits peeled onto same-engine NoOps inserted immediately before
    it -- identical semantics (engine program order), legal ISA encoding.
    """
    n_split = 0
    for f in nc.m.functions:
        for bb in f.blocks:
            new_insts = []
            for inst in bb.instructions:
                si = getattr(inst, "sync_info", None)
                if si is not None and len(si.on_wait) > max_waits:
                    waits = list(si.on_wait)
                    for w in waits[max_waits:]:
                        nop = mybir.InstNoOp(
                            name=nc.get_next_instruction_name(), ins=[], outs=[]
                        )
                        nop.engine = inst.engine
                        nop.sync_info = bass_rust.SyncInfo(
                            on_wait=[w], on_update=[]
                        )
                        new_insts.append(nop)
                        n_split += 1
                    inst.sync_info = bass_rust.SyncInfo(
                        on_wait=waits[:max_waits], on_update=list(si.on_update)
                    )
                new_insts.append(inst)
            bb.instructions = new_insts
    return n_split


def _emit(tc, x, gw, w1, b1, w2, b2, outT):
    nc = tc.nc

    with (
        tc.tile_pool(name="const", bufs=1) as const_pool,
        tc.tile_pool(name="persist", bufs=1) as persist,
        tc.tile_pool(name="wstage", bufs=3) as wstage,
        tc.tile_pool(name="w1pool", bufs=3) as w1pool,
        tc.tile_pool(name="w2pool", bufs=3) as w2pool,
        tc.tile_pool(name="bpool", bufs=4) as bpool,
        tc.tile_pool(name="hpool", bufs=3) as hpool,
        tc.tile_pool(name="sbpool", bufs=3) as sbpool,
        tc.tile_pool(name="fpsum", bufs=4, space="PSUM") as fpsum,
    ):
        ident = const_pool.tile([128, 128], FP, tag="ident")
        make_identity(nc, ident[:])
        ones_f = const_pool.tile([1, 128], FP, tag="ones_f")
        nc.vector.memset(ones_f[:], 1.0)
        ones_row = const_pool.tile([1, 128], BF, tag="ones")
        nc.vector.tensor_copy(ones_row[:], ones_f[:])

        # gate_w [D, E] -> per-d-chunk [128, E] blocks, free-concatenated
        gw_sb = const_pool.tile([128, KD * E], FP, tag="gw")
        for k in range(KD):
            nc.sync.dma_start(
                gw_sb[:, k * E:(k + 1) * E], gw[k * 128:(k + 1) * 128, :]
            )
        # b2 [E, D] natural layout (E on partitions), cast to bf16
        b2f = const_pool.tile([E, D], FP, tag="b2f")
        nc.gpsimd.dma_start(b2f[:], b2[:, :])
        b2T = persist.tile([E, D], BF, tag="b2T")
        nc.vector.tensor_copy(b2T[:], b2f[:])

        # bf16 xT for FFN matmuls; exact fp32 xg (stage-scoped) for gating so
        # the top-2 selection matches the oracle.
        xT = [
            persist.tile([128, TOK], BF, tag=f"xT{d}", name=f"xT{d}")
            for d in range(KD)
        ]
        ST = persist.tile([E, TOK], BF, tag="ST")
        acc = [
            persist.tile([128, TOK], FP, tag=f"acc{m}", name=f"acc{m}")
            for m in range(KD)
        ]

        # weight streaming: DMA on gpsimd (keeps the sync queue free for x),
        # bf16 casts on ACT; prefetched two experts ahead.
        loaded = {}

        def _stage_dma(dram_rows, n_chunks, chunk_free, after=None):
            stgs = []
            for k in range(n_chunks):
                stg = wstage.tile([128, chunk_free], FP, tag="ws", name="stg")
                di = nc.gpsimd.dma_start(
                    stg[:], dram_rows[k * 128:(k + 1) * 128, :]
                )
                if after is not None:
                    # hold this transfer until the prologue x loads finish
                    # so x keeps full HBM bandwidth
                    add_dep_helper(di.ins, after, reason="hbm x-priority")
                stgs.append(stg)
            return stgs

        def _cast_bf16(stgs, chunk_free, dst, use_act):
            # emitted late so these casts never head-of-line-block the
            # current expert's gelu (ACT) / scale (DVE) work
            for k, stg in enumerate(stgs):
                dsl = dst[:, k * chunk_free:(k + 1) * chunk_free]
                if use_act:
                    nc.scalar.copy(dsl, stg[:])
                else:
                    nc.vector.tensor_copy(dsl, stg[:])

        def _load_w1(e, after=None):
            w1t = w1pool.tile([128, KD * H], BF, tag="w1", name="w1t")
            _cast_bf16(
                _stage_dma(w1[e], KD, H, after=after), H, w1t, use_act=True
            )
            b1t = bpool.tile([128, KH], FP, tag="b1", name="b1t")
            nc.gpsimd.dma_start(b1t[:], b1[e].rearrange("(k p) -> p k", p=128))
            loaded[e] = (w1t, b1t)

        def _load_w2(e):
            w2t = w2pool.tile([128, KH * D], BF, tag="w2", name="w2t")
            _cast_bf16(_stage_dma(w2[e], KH, D), D, w2t, use_act=False)
            loaded_w2[e] = w2t

        def _l1_both(w1t, b1t, hts, sbt):
            # both token halves per (mh, kd) so each stationary tile is used
            # by two back-to-back matmuls (reload hidden / elided)
            for mh in range(KH):
                ph = [
                    fpsum.tile([128, NF], FP, tag="ph", name="ph")
                    for _ in range(NHF)
                ]
                for kd in range(KD):
                    stat = w1t[:, kd * H + mh * 128:kd * H + (mh + 1) * 128]
                    for hf in range(NHF):
                        nc.tensor.matmul(
                            ph[hf][:],
                            stat,
                            xT[kd][:, hf * NF:(hf + 1) * NF],
                            start=(kd == 0),
                            stop=(kd == KD - 1),
                        )
                for hf in range(NHF):
                    hsl = hts[:, mh * TOK + hf * NF:mh * TOK + (hf + 1) * NF]
                    nc.scalar.activation(
                        hsl, ph[hf][:], AF.Gelu, bias=b1t[:, mh:mh + 1]
                    )
                    nc.vector.tensor_tensor(
                        hsl, hsl, sbt[:, hf * NF:(hf + 1) * NF], op=ALU.mult
                    )

        def _l1_half(w1t, b1t, hts, hf, sbt=None):
            # layer 1 for one token half: hts[:, mh, hf] = gelu(w1.T @ xT + b1)
            # scaled by the expert's per-token gate weight when sbt is given
            for mh in range(KH):
                ph = fpsum.tile([128, NF], FP, tag="ph", name="ph")
                for kd in range(KD):
                    nc.tensor.matmul(
                        ph[:],
                        w1t[:, kd * H + mh * 128:kd * H + (mh + 1) * 128],
                        xT[kd][:, hf * NF:(hf + 1) * NF],
                        start=(kd == 0),
                        stop=(kd == KD - 1),
                    )
                hsl = hts[:, mh * TOK + hf * NF:mh * TOK + (hf + 1) * NF]
                nc.scalar.activation(hsl, ph[:], AF.Gelu, bias=b1t[:, mh:mh + 1])
                if sbt is not None:
                    nc.vector.tensor_tensor(
                        hsl, hsl, sbt[:, hf * NF:(hf + 1) * NF], op=ALU.mult
                    )

        loaded_w2 = {}
        xlast = {}
        hts_pair = {}
        w2_pair = {}

        # ---- stage 1: x load + transpose + gating, with expert-0 layer 1
        # interleaved so the PE stream stays dense (HAM warm) ---------------
        with (
            tc.tile_pool(name="xin", bufs=4) as xin_pool,
            tc.tile_pool(name="xg", bufs=1) as xg_pool,
            tc.tile_pool(name="tpsum", bufs=2, space="PSUM") as tpsum,
            tc.tile_pool(name="gpsum", bufs=1, space="PSUM") as gpsum,
            tc.tile_pool(name="gtmp", bufs=3) as gtmp,
        ):
            xg = [
                xg_pool.tile([128, TOK], FP, tag=f"xg{d}", name=f"xg{d}")
                for d in range(KD)
            ]

            def _tchunk(t):
                xt = xin_pool.tile([128, D], FP, tag="xt", name="xt")
                # split the 512KB tile load across 8 DMA queues on all
                # three DMA-capable engines (one queue only ~50 GB/s)
                engs = [nc.sync, nc.scalar, nc.gpsimd]
                for q in range(8):
                    di = engs[q % 3].dma_start(
                        xt[:, q * (D // 8):(q + 1) * (D // 8)],
                        x[t * 128:(t + 1) * 128,
                          q * (D // 8):(q + 1) * (D // 8)],
                    )
                    xlast[t] = di.ins
                for d in range(KD):
                    pt = tpsum.tile([128, 128], FP, tag="pt", name="pt")
                    nc.tensor.transpose(
                        pt[:], xt[:, d * 128:(d + 1) * 128], ident[:]
                    )
                    nc.vector.tensor_copy(
                        xg[d][:, t * 128:(t + 1) * 128], pt[:]
                    )
                    nc.vector.tensor_copy(
                        xT[d][:, t * 128:(t + 1) * 128], pt[:]
                    )
                # gating for this token chunk (exact fp32)
                ts = slice(t * 128, (t + 1) * 128)
                pg = gpsum.tile([128, E], FP, tag="pg", name="pg")
                for d in range(KD):
                    nc.tensor.matmul(
                        pg[:],
                        xg[d][:, ts],
                        gw_sb[:, d * E:(d + 1) * E],
                        start=(d == 0),
                        stop=(d == KD - 1),
                    )
                m = gtmp.tile([128, 1], FP, tag="m", name="m")
                nc.vector.tensor_reduce(m[:], pg[:], axis=AX.X, op=ALU.max)
                nm = gtmp.tile([128, 1], FP, tag="nm", name="nm")
                nc.vector.tensor_scalar(nm[:], m[:], -1.0, None, op0=ALU.mult)
                ex = gtmp.tile([128, E], FP, tag="ex", name="ex")
                nc.scalar.activation(ex[:], pg[:], AF.Exp, bias=nm[:, 0:1])
                ssum = gtmp.tile([128, 1], FP, tag="ssum", name="ssum")
                nc.vector.tensor_reduce(ssum[:], ex[:], axis=AX.X, op=ALU.add)
                r = gtmp.tile([128, 1], FP, tag="r", name="r")
                nc.vector.reciprocal(r[:], ssum[:])
                g = gtmp.tile([128, E], FP, tag="g", name="g")
                nc.vector.tensor_scalar(g[:], ex[:], r[:, 0:1], None, op0=ALU.mult)
                # top-2: m1 = max, m2 = max after suppressing the argmax
                m1 = gtmp.tile([128, 1], FP, tag="m1", name="m1")
                nc.vector.tensor_reduce(m1[:], g[:], axis=AX.X, op=ALU.max)
                is1 = gtmp.tile([128, E], FP, tag="is1", name="is1")
                nc.vector.tensor_scalar(
                    is1[:], g[:], m1[:, 0:1], None, op0=ALU.is_ge
                )
                g2 = gtmp.tile([128, E], FP, tag="g2", name="g2")
                nc.vector.tensor_scalar(g2[:], is1[:], -2.0, None, op0=ALU.mult)
                nc.vector.tensor_tensor(g2[:], g2[:], g[:], op=ALU.add)
                m2 = gtmp.tile([128, 1], FP, tag="m2", name="m2")
                nc.vector.tensor_reduce(m2[:], g2[:], axis=AX.X, op=ALU.max)
                tokw = gtmp.tile([128, 1], FP, tag="tokw", name="tokw")
                nc.vector.tensor_tensor(tokw[:], m1[:], m2[:], op=ALU.add)
                sel = gtmp.tile([128, E], FP, tag="sel", name="sel")
                nc.vector.tensor_scalar(
                    sel[:], g[:], m2[:, 0:1], None, op0=ALU.is_ge
                )
                sw = gtmp.tile([128, E], FP, tag="sw", name="sw")
                nc.vector.tensor_scalar(
                    sw[:], sel[:], tokw[:, 0:1], None, op0=ALU.mult
                )
                # transpose S chunk [128, E] -> ST[:, t*128:+128] (bf16)
                pst = gpsum.tile([128, 128], FP, tag="pst", name="pst")
                nc.tensor.transpose(pst[0:E, :], sw[:], ident[:])
                nc.vector.tensor_copy(ST[:, ts], pst[0:E, :])

            # six chunks of transposes+gating give the PE dense work while
            # x streams at full HBM bandwidth; w1[0] transfers only start
            # once the first-half x chunks are in (dep edge), so expert 0's
            # layer 1 lands just-in-time after chunk 5
            for t in range(6):
                _tchunk(t)
            _load_w1(0, after=xlast[3])
            hts0 = hpool.tile([128, KH * TOK], BF, tag="h", name="hts0")
            hts_pair[0] = hts0
            _l1_half(loaded[0][0], loaded[0][1], hts0, 0)
            _tchunk(6)
            _tchunk(7)
            _load_w1(1, after=xlast[5])
            _l1_half(loaded[0][0], loaded[0][1], hts0, 1)

        # ---- stage 2: per-expert FFN (bf16), expert-pair PSUM accum -------
        with (
            tc.tile_pool(name="bpsum", bufs=1, space="PSUM") as bpsum,
            tc.tile_pool(name="ypsum", bufs=3, space="PSUM") as ypsum,
        ):
            def _sbt_for(e):
                # expert's S row to partition 0, then broadcast to all 128
                # partitions via a K=1 ones-matmul
                ste = sbpool.tile([1, TOK], BF, tag="ste", name="ste")
                nc.sync.dma_start(ste[:], ST[e:e + 1, :])
                sbt = sbpool.tile([128, TOK], BF, tag="sb", name="sbt")
                for hf in range(NHF):
                    pb = bpsum.tile([128, NF], FP, tag="pb", name="pb")
                    nc.tensor.matmul(
                        pb[:],
                        ones_row[:],
                        ste[:, hf * NF:(hf + 1) * NF],
                        start=True,
                        stop=True,
                    )
                    nc.vector.tensor_copy(sbt[:, hf * NF:(hf + 1) * NF], pb[:])
                return sbt

            def _scale(hts, sbt):
                for mh in range(KH):
                    for hf in range(NHF):
                        hsl = hts[
                            :, mh * TOK + hf * NF:mh * TOK + (hf + 1) * NF
                        ]
                        nc.vector.tensor_tensor(
                            hsl, hsl, sbt[:, hf * NF:(hf + 1) * NF],
                            op=ALU.mult,
                        )

            for e in range(E):
                eo = e % 2
                if e + 2 < E:
                    _load_w1(e + 2)
                w1t, b1t = loaded.pop(e)
                sbt = _sbt_for(e)
                if e > 0:
                    hts = hpool.tile([128, KH * TOK], BF, tag="h", name="hts")
                    hts_pair[eo] = hts
                    _l1_both(w1t, b1t, hts, sbt)
                else:
                    _scale(hts_pair[eo], sbt)
                _load_w2(e)
                w2_pair[eo] = loaded_w2.pop(e)
                if eo == 0:
                    continue
                # layer 2 for the expert pair (e-1, e), PSUM-accumulated
                for md in range(KD):
                    py = [
                        ypsum.tile([128, NF], FP, tag="py", name="py")
                        for _ in range(NHF)
                    ]
                    for po in (0, 1):
                        for kh in range(KH):
                            stat = w2_pair[po][
                                :, kh * D + md * 128:kh * D + (md + 1) * 128
                            ]
                            for hf in range(NHF):
                                nc.tensor.matmul(
                                    py[hf][:],
                                    stat,
                                    hts_pair[po][
                                        :,
                                        kh * TOK + hf * NF:kh * TOK + (hf + 1) * NF,
                                    ],
                                    start=(po == 0 and kh == 0),
                                    stop=(po == 1 and kh == KH - 1 and e != 1),
                                )
                    if e == 1:
                        # + sum_e S_e[tok] * b2[e, d] as a rank-8 matmul
                        for hf in range(NHF):
                            nc.tensor.matmul(
                                py[hf][:],
                                b2T[:, md * 128:(md + 1) * 128],
                                ST[:, hf * NF:(hf + 1) * NF],
                                start=False,
                                stop=True,
                            )
                    for hf in range(NHF):
                        asl = acc[md][:, hf * NF:(hf + 1) * NF]
                        if e == 1:
                            nc.vector.tensor_copy(asl, py[hf][:])
                        else:
                            nc.vector.tensor_tensor(
                                asl, asl, py[hf][:], op=ALU.add
                            )
                        if e == E - 1 and hf == NHF - 1:
                            # final pair: stream this d-chunk out right away,
                            # split across both HWDGE engines
                            eng = nc.sync if md % 2 == 0 else nc.scalar
                            eng.dma_start(
                                outT[md * 128:(md + 1) * 128, :TOK // 2],
                                acc[md][:, :TOK // 2],
                            )
                            eng2 = nc.scalar if md % 2 == 0 else nc.sync
                            eng2.dma_start(
                                outT[md * 128:(md + 1) * 128, TOK // 2:],
                                acc[md][:, TOK // 2:],
                            )


_CACHED_NC = None


def _build():
    global _CACHED_NC
    if _CACHED_NC is not None:
        return _CACHED_NC
    nc = bass.Bass(
        "TRN2", target_bir_lowering=False, debug=False, num_devices=N_CORES
    )
    x = nc.dram_tensor("x", [TOK, D], FP, kind="ExternalInput").ap()
    gw = nc.dram_tensor("gate_w", [D, E], FP, kind="ExternalInput").ap()
    w1 = nc.dram_tensor("w1", [E, D, H], FP, kind="ExternalInput").ap()
    b1 = nc.dram_tensor("b1", [E, H], FP, kind="ExternalInput").ap()
    w2 = nc.dram_tensor("w2", [E, H, D], FP, kind="ExternalInput").ap()
    b2 = nc.dram_tensor("b2", [E, D], FP, kind="ExternalInput").ap()
    outT = nc.dram_tensor("outT", [D, TOK], FP, kind="ExternalOutput").ap()
    with tile.TileContext(nc) as tc:
        _emit(tc, x, gw, w1, b1, w2, b2, outT)
    _legalize_sync_waits(nc)
    _CACHED_NC = nc
    return nc


def run(inputs, **spmd_kwargs):
    """Shard, run on 8 cores, unshard. Returns (out [B,S,D], BassKernelResults)."""
    nc = _build()
    xf = np.ascontiguousarray(
        np.asarray(inputs["x"], dtype=np.float32).reshape(NTOK, D)
    )
    shared = {
        k: np.ascontiguousarray(np.asarray(inputs[k], dtype=np.float32))
        for k in ("gate_w", "w1", "b1", "w2", "b2")
    }
    in_maps = [
        {"x": xf[c * TOK:(c + 1) * TOK], **shared} for c in range(N_CORES)
    ]
    res = run_bass_kernel_spmd(nc, in_maps, list(range(N_CORES)), **spmd_kwargs)
    out = np.concatenate(
        [res.results[c]["outT"].T for c in range(N_CORES)], axis=0
    )
    return out.reshape(B, S, D).astype(np.float32, copy=False), res


def kernel(**inputs):
    out, _ = run(inputs)
    return out



# revision 36
# speedup vs baseline: 1.0468x; 1.0468x over previous
"""MoE FFN (EnterpriseFFN) Trainium2 kernel.

8192 tokens x d_model=1024, 8 experts (hidden 512), top-2 gating where every
selected expert is scaled by the SUM of the top-2 softmax gates.

Distribution: data-parallel over tokens -- each of the 8 NeuronCores runs
1024 tokens through all 8 experts (dense compute, masked combine, exactly
like the reference einsum formulation). Expert weights are replicated.

Per-core pipeline (activations kept transposed, [feature, token]):
  1. Load x [1024 tok, 1024 d]; PE-transpose to fp32 xg (gating) and bf16 xT
     (FFN) tiles, with per-chunk gating (softmax + top-2 via max / masked-max
     on DVE, exact fp32 logits so the top-2 selection matches the oracle);
     S[tok, e] = sel * tok_w is PE-transposed to ST [e, tok]. Expert 0's
     layer 1 is interleaved to keep the PE stream dense (HAM warm).
  2. Per expert e: hT = gelu(w1[e].T-chunks @ xT + b1) on PE/ACT (bf16 in,
     fp32 PSUM), scaled along tokens by a ones-matmul broadcast of ST's row;
     layer 2 accumulates expert PAIRS plus the rank-8 b2 @ S matmul in PSUM;
     a fp32 SBUF accumulator sums the pairs.
  3. Store yT [d, tok]; the host transposes shards back and concatenates.

FFN matmuls run in bf16 (fast weight load, 1 cyc/row); gating runs in exact
fp32. Weight tiles are DMA-staged fp32 then cast to bf16 on ACT/DVE.
"""

import numpy as np

import bass_rust
import concourse.bass as bass
import concourse.tile as tile
from concourse import mybir
from concourse.bass_utils import run_bass_kernel_spmd
from concourse.masks import make_identity
from concourse.tile_rust import add_dep_helper

N_CORES = 8
B, S, D, H, E = 4, 2048, 1024, 512, 8
NTOK = B * S          # 8192 total tokens
TOK = NTOK // N_CORES  # 1024 tokens per core
KD = D // 128          # 8 d_model chunks
KH = H // 128          # 4 hidden chunks
TT = TOK // 128        # 8 token chunks
NF = 512               # matmul moving free width
NHF = TOK // NF        # 2 token halves

FP = mybir.dt.float32
BF = mybir.dt.bfloat16
AF = mybir.ActivationFunctionType
ALU = mybir.AluOpType
AX = mybir.AxisListType


def _legalize_sync_waits(nc, max_waits=1):
    """Split multi-wait instructions for this walrus (1 sync wait per inst).

    Any instruction carrying more than ``max_waits`` sync-wait commands gets
    the extra waits peeled onto same-engine NoOps inserted immediately before
    it -- identical semantics (engine program order), legal ISA encoding.
    """
    n_split = 0
    for f in nc.m.functions:
        for bb in f.blocks:
            new_insts = []
            for inst in bb.instructions:
                si = getattr(inst, "sync_info", None)
                if si is not None and len(si.on_wait) > max_waits:
                    waits = list(si.on_wait)
                    for w in waits[max_waits:]:
                        nop = mybir.InstNoOp(
                            name=nc.get_next_instruction_name(), ins=[], outs=[]
                        )
                        nop.engine = inst.engine
                        nop.sync_info = bass_rust.SyncInfo(
                            on_wait=[w], on_update=[]
                        )
                        new_insts.append(nop)
                        n_split += 1
                    inst.sync_info = bass_rust.SyncInfo(
                        on_wait=waits[:max_waits], on_update=list(si.on_update)
                    )
                new_insts.append(inst)
            bb.instructions = new_insts
    return n_split


def _emit(tc, x, gw, w1, b1, w2, b2, outT):
    nc = tc.nc

    with (
        tc.tile_pool(name="const", bufs=1) as const_pool,
        tc.tile_pool(name="persist", bufs=1) as persist,
        tc.tile_pool(name="wstage", bufs=3) as wstage,
        tc.tile_pool(name="w1pool", bufs=3) as w1pool,
        tc.tile_pool(name="w2pool", bufs=3) as w2pool,
        tc.tile_pool(name="bpool", bufs=4) as bpool,
        tc.tile_pool(name="hpool", bufs=3) as hpool,
        tc.tile_pool(name="sbpool", bufs=3) as sbpool,
        tc.tile_pool(name="fpsum", bufs=3, space="PSUM") as fpsum,
    ):
        ident = const_pool.tile([128, 128], FP, tag="ident")
        make_identity(nc, ident[:])
        ones_f = const_pool.tile([1, 128], FP, tag="ones_f")
        nc.vector.memset(ones_f[:], 1.0)
        ones_row = const_pool.tile([1, 128], BF, tag="ones")
        nc.vector.tensor_copy(ones_row[:], ones_f[:])

        # gate_w [D, E] -> per-d-chunk [128, E] blocks, free-concatenated.
        # These are 32B-per-partition strided loads (1024 tiny descriptors);
        # they are emitted AFTER the x chunks so they never head-of-line
        # block the x stream, and gating only needs them ~40us in.
        gw_sb = const_pool.tile([128, KD * E], FP, tag="gw")
        # b2 [E, D] natural layout (E on partitions), cast to bf16
        b2f = const_pool.tile([E, D], FP, tag="b2f")
        nc.gpsimd.dma_start(b2f[:], b2[:, :])
        b2T = persist.tile([E, D], BF, tag="b2T")
        nc.vector.tensor_copy(b2T[:], b2f[:])
        # b1 [E, H] loaded whole (4 descriptors) and PE-transposed into the
        # per-partition bias layout b1t_all[:, mh*E+e] -- replaces 8 per-
        # expert strided loads of 512 x 4B descriptors each.
        b1nat = const_pool.tile([E, H], FP, tag="b1nat")
        nc.gpsimd.dma_start(b1nat[:], b1[:, :])
        b1t_all = persist.tile([128, KH * E], FP, tag="b1t_all")

        # bf16 xT for FFN matmuls; exact fp32 xg (stage-scoped) for gating so
        # the top-2 selection matches the oracle.
        xT = [
            persist.tile([128, TOK], BF, tag=f"xT{d}", name=f"xT{d}")
            for d in range(KD)
        ]
        ST = persist.tile([E, TOK], BF, tag="ST")
        acc = [
            persist.tile([128, TOK], FP, tag=f"acc{m}", name=f"acc{m}")
            for m in range(KD)
        ]

        # weight streaming: DMA on gpsimd (keeps the sync queue free for x),
        # bf16 casts on ACT; prefetched two experts ahead.
        loaded = {}

        def _stage_dma(dram_rows, n_chunks, chunk_free, after=None):
            stgs = []
            for k in range(n_chunks):
                stg = wstage.tile([128, chunk_free], FP, tag="ws", name="stg")
                di = nc.gpsimd.dma_start(
                    stg[:], dram_rows[k * 128:(k + 1) * 128, :]
                )
                if after is not None:
                    # hold this transfer until the prologue x loads finish
                    # so x keeps full HBM bandwidth
                    add_dep_helper(di.ins, after, reason="hbm x-priority")
                stgs.append(stg)
            return stgs

        def _cast_bf16(stgs, chunk_free, dst, use_act):
            # emitted late so these casts never head-of-line-block the
            # current expert's gelu (ACT) / scale (DVE) work
            for k, stg in enumerate(stgs):
                dsl = dst[:, k * chunk_free:(k + 1) * chunk_free]
                if use_act:
                    nc.scalar.copy(dsl, stg[:])
                else:
                    nc.vector.tensor_copy(dsl, stg[:])

        def _load_w1(e, after=None):
            w1t = w1pool.tile([128, KD * H], BF, tag="w1", name="w1t")
            _cast_bf16(
                _stage_dma(w1[e], KD, H, after=after), H, w1t, use_act=True
            )
            loaded[e] = (w1t, e)

        def _load_w2(e):
            w2t = w2pool.tile([128, KH * D], BF, tag="w2", name="w2t")
            _cast_bf16(_stage_dma(w2[e], KH, D), D, w2t, use_act=False)
            loaded_w2[e] = w2t

        def _l1_both(w1t, b1e, hts, sbt):
            # both token halves per (mh, kd) so each stationary tile is used
            # by two back-to-back matmuls (reload hidden / elided)
            for mh in range(KH):
                ph = [
                    fpsum.tile([128, NF], FP, tag="ph", name="ph")
                    for _ in range(NHF)
                ]
                for kd in range(KD):
                    stat = w1t[:, kd * H + mh * 128:kd * H + (mh + 1) * 128]
                    for hf in range(NHF):
                        nc.tensor.matmul(
                            ph[hf][:],
                            stat,
                            xT[kd][:, hf * NF:(hf + 1) * NF],
                            start=(kd == 0),
                            stop=(kd == KD - 1),
                        )
                for hf in range(NHF):
                    hsl = hts[:, mh * TOK + hf * NF:mh * TOK + (hf + 1) * NF]
                    nc.scalar.activation(
                        hsl, ph[hf][:], AF.Gelu,
                        bias=b1t_all[:, mh * E + b1e:mh * E + b1e + 1],
                    )
                    nc.vector.tensor_tensor(
                        hsl, hsl, sbt[:, hf * NF:(hf + 1) * NF], op=ALU.mult
                    )

        def _l1_half(w1t, b1e, hts, hf, sbt=None):
            # layer 1 for one token half: hts[:, mh, hf] = gelu(w1.T @ xT + b1)
            # scaled by the expert's per-token gate weight when sbt is given
            for mh in range(KH):
                ph = fpsum.tile([128, NF], FP, tag="ph", name="ph")
                for kd in range(KD):
                    nc.tensor.matmul(
                        ph[:],
                        w1t[:, kd * H + mh * 128:kd * H + (mh + 1) * 128],
                        xT[kd][:, hf * NF:(hf + 1) * NF],
                        start=(kd == 0),
                        stop=(kd == KD - 1),
                    )
                hsl = hts[:, mh * TOK + hf * NF:mh * TOK + (hf + 1) * NF]
                nc.scalar.activation(
                    hsl, ph[:], AF.Gelu,
                    bias=b1t_all[:, mh * E + b1e:mh * E + b1e + 1],
                )
                if sbt is not None:
                    nc.vector.tensor_tensor(
                        hsl, hsl, sbt[:, hf * NF:(hf + 1) * NF], op=ALU.mult
                    )

        loaded_w2 = {}
        xlast = {}
        hts_pair = {}
        w2_pair = {}

        # ---- stage 1: x load + transpose + gating, with expert-0 layer 1
        # interleaved so the PE stream stays dense (HAM warm) ---------------
        with (
            tc.tile_pool(name="xin", bufs=4) as xin_pool,
            tc.tile_pool(name="xg", bufs=1) as xg_pool,
            tc.tile_pool(name="tpsum", bufs=2, space="PSUM") as tpsum,
            tc.tile_pool(name="gpsum", bufs=1, space="PSUM") as gpsum,
            tc.tile_pool(name="gtmp", bufs=3) as gtmp,
        ):
            xg = [
                xg_pool.tile([128, TOK], FP, tag=f"xg{d}", name=f"xg{d}")
                for d in range(KD)
            ]

            def _tchunk(t):
                xt = xin_pool.tile([128, D], FP, tag="xt", name="xt")
                # split the 512KB tile load across 8 DMA queues on all
                # three DMA-capable engines (one queue only ~50 GB/s)
                engs = [nc.sync, nc.scalar, nc.gpsimd]
                for q in range(8):
                    di = engs[q % 3].dma_start(
                        xt[:, q * (D // 8):(q + 1) * (D // 8)],
                        x[t * 128:(t + 1) * 128,
                          q * (D // 8):(q + 1) * (D // 8)],
                    )
                    xlast[t] = di.ins
                for d in range(KD):
                    pt = tpsum.tile([128, 128], FP, tag="pt", name="pt")
                    nc.tensor.transpose(
                        pt[:], xt[:, d * 128:(d + 1) * 128], ident[:]
                    )
                    nc.vector.tensor_copy(
                        xg[d][:, t * 128:(t + 1) * 128], pt[:]
                    )
                    nc.vector.tensor_copy(
                        xT[d][:, t * 128:(t + 1) * 128], pt[:]
                    )
                # gating for this token chunk (exact fp32)
                ts = slice(t * 128, (t + 1) * 128)
                pg = gpsum.tile([128, E], FP, tag="pg", name="pg")
                for d in range(KD):
                    nc.tensor.matmul(
                        pg[:],
                        xg[d][:, ts],
                        gw_sb[:, d * E:(d + 1) * E],
                        start=(d == 0),
                        stop=(d == KD - 1),
                    )
                m = gtmp.tile([128, 1], FP, tag="m", name="m")
                nc.vector.tensor_reduce(m[:], pg[:], axis=AX.X, op=ALU.max)
                nm = gtmp.tile([128, 1], FP, tag="nm", name="nm")
                nc.vector.tensor_scalar(nm[:], m[:], -1.0, None, op0=ALU.mult)
                ex = gtmp.tile([128, E], FP, tag="ex", name="ex")
                nc.scalar.activation(ex[:], pg[:], AF.Exp, bias=nm[:, 0:1])
                ssum = gtmp.tile([128, 1], FP, tag="ssum", name="ssum")
                nc.vector.tensor_reduce(ssum[:], ex[:], axis=AX.X, op=ALU.add)
                r = gtmp.tile([128, 1], FP, tag="r", name="r")
                nc.vector.reciprocal(r[:], ssum[:])
                g = gtmp.tile([128, E], FP, tag="g", name="g")
                nc.vector.tensor_scalar(g[:], ex[:], r[:, 0:1], None, op0=ALU.mult)
                # top-2: m1 = max, m2 = max after suppressing the argmax
                m1 = gtmp.tile([128, 1], FP, tag="m1", name="m1")
                nc.vector.tensor_reduce(m1[:], g[:], axis=AX.X, op=ALU.max)
                is1 = gtmp.tile([128, E], FP, tag="is1", name="is1")
                nc.vector.tensor_scalar(
                    is1[:], g[:], m1[:, 0:1], None, op0=ALU.is_ge
                )
                g2 = gtmp.tile([128, E], FP, tag="g2", name="g2")
                nc.vector.tensor_scalar(g2[:], is1[:], -2.0, None, op0=ALU.mult)
                nc.vector.tensor_tensor(g2[:], g2[:], g[:], op=ALU.add)
                m2 = gtmp.tile([128, 1], FP, tag="m2", name="m2")
                nc.vector.tensor_reduce(m2[:], g2[:], axis=AX.X, op=ALU.max)
                tokw = gtmp.tile([128, 1], FP, tag="tokw", name="tokw")
                nc.vector.tensor_tensor(tokw[:], m1[:], m2[:], op=ALU.add)
                sel = gtmp.tile([128, E], FP, tag="sel", name="sel")
                nc.vector.tensor_scalar(
                    sel[:], g[:], m2[:, 0:1], None, op0=ALU.is_ge
                )
                sw = gtmp.tile([128, E], FP, tag="sw", name="sw")
                nc.vector.tensor_scalar(
                    sw[:], sel[:], tokw[:, 0:1], None, op0=ALU.mult
                )
                # transpose S chunk [128, E] -> ST[:, t*128:+128] (bf16)
                pst = gpsum.tile([128, 128], FP, tag="pst", name="pst")
                nc.tensor.transpose(pst[0:E, :], sw[:], ident[:])
                nc.vector.tensor_copy(ST[:, ts], pst[0:E, :])

            for mh in range(KH):
                ptb = tpsum.tile([128, 128], FP, tag="pt", name="ptb")
                nc.tensor.transpose(
                    ptb[:, 0:E], b1nat[:, mh * 128:(mh + 1) * 128],
                    ident[0:E, 0:E],
                )
                nc.vector.tensor_copy(
                    b1t_all[:, mh * E:(mh + 1) * E], ptb[:, 0:E]
                )
            for k in range(KD):
                eng = nc.sync if k % 2 == 0 else nc.scalar
                eng.dma_start(
                    gw_sb[:, k * E:(k + 1) * E], gw[k * 128:(k + 1) * 128, :]
                )
            # six chunks of transposes+gating give the PE dense work while
            # x streams at full HBM bandwidth; w1[0] transfers only start
            # once the first-half x chunks are in (dep edge), so expert 0's
            # layer 1 lands just-in-time after chunk 5
            for t in range(6):
                _tchunk(t)
            _load_w1(0, after=xlast[3])
            hts0 = hpool.tile([128, KH * TOK], BF, tag="h", name="hts0")
            hts_pair[0] = hts0
            _l1_half(loaded[0][0], loaded[0][1], hts0, 0)
            _tchunk(6)
            _tchunk(7)
            _load_w1(1, after=xlast[5])
            _l1_half(loaded[0][0], loaded[0][1], hts0, 1)

        # ---- stage 2: per-expert FFN (bf16), expert-pair PSUM accum -------
        with (
            tc.tile_pool(name="bpsum", bufs=1, space="PSUM") as bpsum,
            tc.tile_pool(name="ypsum", bufs=4, space="PSUM") as ypsum,
        ):
            def _sbt_for(e):
                # expert's S row to partition 0, then broadcast to all 128
                # partitions via a K=1 ones-matmul
                ste = sbpool.tile([1, TOK], BF, tag="ste", name="ste")
                nc.sync.dma_start(ste[:], ST[e:e + 1, :])
                sbt = sbpool.tile([128, TOK], BF, tag="sb", name="sbt")
                for hf in range(NHF):
                    pb = bpsum.tile([128, NF], FP, tag="pb", name="pb")
                    nc.tensor.matmul(
                        pb[:],
                        ones_row[:],
                        ste[:, hf * NF:(hf + 1) * NF],
                        start=True,
                        stop=True,
                    )
                    nc.vector.tensor_copy(sbt[:, hf * NF:(hf + 1) * NF], pb[:])
                return sbt

            def _scale(hts, sbt):
                for mh in range(KH):
                    for hf in range(NHF):
                        hsl = hts[
                            :, mh * TOK + hf * NF:mh * TOK + (hf + 1) * NF
                        ]
                        nc.vector.tensor_tensor(
                            hsl, hsl, sbt[:, hf * NF:(hf + 1) * NF],
                            op=ALU.mult,
                        )

            for e in range(E):
                eo = e % 2
                if e + 2 < E:
                    _load_w1(e + 2)
                w1t, b1t = loaded.pop(e)
                sbt = _sbt_for(e)
                if e > 0:
                    hts = hpool.tile([128, KH * TOK], BF, tag="h", name="hts")
                    hts_pair[eo] = hts
                    _l1_both(w1t, b1t, hts, sbt)
                else:
                    _scale(hts_pair[eo], sbt)
                _load_w2(e)
                w2_pair[eo] = loaded_w2.pop(e)
                if eo == 0:
                    continue
                # layer 2 for the expert pair (e-1, e), PSUM-accumulated
                for md in range(KD):
                    py = [
                        ypsum.tile([128, NF], FP, tag="py", name="py")
                        for _ in range(NHF)
                    ]
                    for po in (0, 1):
                        for kh in range(KH):
                            stat = w2_pair[po][
                                :, kh * D + md * 128:kh * D + (md + 1) * 128
                            ]
                            for hf in range(NHF):
                                nc.tensor.matmul(
                                    py[hf][:],
                                    stat,
                                    hts_pair[po][
                                        :,
                                        kh * TOK + hf * NF:kh * TOK + (hf + 1) * NF,
                                    ],
                                    start=(po == 0 and kh == 0),
                                    stop=(po == 1 and kh == KH - 1 and e != 1),
                                )
                    if e == 1:
                        # + sum_e S_e[tok] * b2[e, d] as a rank-8 matmul
                        for hf in range(NHF):
                            nc.tensor.matmul(
                                py[hf][:],
                                b2T[:, md * 128:(md + 1) * 128],
                                ST[:, hf * NF:(hf + 1) * NF],
                                start=False,
                                stop=True,
                            )
                    for hf in range(NHF):
                        asl = acc[md][:, hf * NF:(hf + 1) * NF]
                        if e == 1:
                            nc.vector.tensor_copy(asl, py[hf][:])
                        else:
                            nc.vector.tensor_tensor(
                                asl, asl, py[hf][:], op=ALU.add
                            )
                        if e == E - 1 and hf == NHF - 1:
                            # final pair: stream this d-chunk out right away,
                            # split across both HWDGE engines
                            eng = nc.sync if md % 2 == 0 else nc.scalar
                            eng.dma_start(
                                outT[md * 128:(md + 1) * 128, :TOK // 2],
                                acc[md][:, :TOK // 2],
                            )
                            eng2 = nc.scalar if md % 2 == 0 else nc.sync
                            eng2.dma_start(
                                outT[md * 128:(md + 1) * 128, TOK // 2:],
                                acc[md][:, TOK // 2:],
                            )


_CACHED_NC = None


def _build():
    global _CACHED_NC
    if _CACHED_NC is not None:
        return _CACHED_NC
    nc = bass.Bass(
        "TRN2", target_bir_lowering=False, debug=False, num_devices=N_CORES
    )
    x = nc.dram_tensor("x", [TOK, D], FP, kind="ExternalInput").ap()
    gw = nc.dram_tensor("gate_w", [D, E], FP, kind="ExternalInput").ap()
    w1 = nc.dram_tensor("w1", [E, D, H], FP, kind="ExternalInput").ap()
    b1 = nc.dram_tensor("b1", [E, H], FP, kind="ExternalInput").ap()
    w2 = nc.dram_tensor("w2", [E, H, D], FP, kind="ExternalInput").ap()
    b2 = nc.dram_tensor("b2", [E, D], FP, kind="ExternalInput").ap()
    outT = nc.dram_tensor("outT", [D, TOK], FP, kind="ExternalOutput").ap()
    with tile.TileContext(nc) as tc:
        _emit(tc, x, gw, w1, b1, w2, b2, outT)
    _legalize_sync_waits(nc)
    _CACHED_NC = nc
    return nc


def run(inputs, **spmd_kwargs):
    """Shard, run on 8 cores, unshard. Returns (out [B,S,D], BassKernelResults)."""
    nc = _build()
    xf = np.ascontiguousarray(
        np.asarray(inputs["x"], dtype=np.float32).reshape(NTOK, D)
    )
    shared = {
        k: np.ascontiguousarray(np.asarray(inputs[k], dtype=np.float32))
        for k in ("gate_w", "w1", "b1", "w2", "b2")
    }
    in_maps = [
        {"x": xf[c * TOK:(c + 1) * TOK], **shared} for c in range(N_CORES)
    ]
    res = run_bass_kernel_spmd(nc, in_maps, list(range(N_CORES)), **spmd_kwargs)
    out = np.concatenate(
        [res.results[c]["outT"].T for c in range(N_CORES)], axis=0
    )
    return out.reshape(B, S, D).astype(np.float32, copy=False), res


def kernel(**inputs):
    out, _ = run(inputs)
    return out



# revision 37
# speedup vs baseline: 1.0534x; 1.0063x over previous
"""MoE FFN (EnterpriseFFN) Trainium2 kernel.

8192 tokens x d_model=1024, 8 experts (hidden 512), top-2 gating where every
selected expert is scaled by the SUM of the top-2 softmax gates.

Distribution: data-parallel over tokens -- each of the 8 NeuronCores runs
1024 tokens through all 8 experts (dense compute, masked combine, exactly
like the reference einsum formulation). Expert weights are replicated.

Per-core pipeline (activations kept transposed, [feature, token]):
  1. Load x [1024 tok, 1024 d]; PE-transpose to fp32 xg (gating) and bf16 xT
     (FFN) tiles, with per-chunk gating (softmax + top-2 via max / masked-max
     on DVE, exact fp32 logits so the top-2 selection matches the oracle);
     S[tok, e] = sel * tok_w is PE-transposed to ST [e, tok]. Expert 0's
     layer 1 is interleaved to keep the PE stream dense (HAM warm).
  2. Per expert e: hT = gelu(w1[e].T-chunks @ xT + b1) on PE/ACT (bf16 in,
     fp32 PSUM), scaled along tokens by a ones-matmul broadcast of ST's row;
     layer 2 accumulates expert PAIRS plus the rank-8 b2 @ S matmul in PSUM;
     a fp32 SBUF accumulator sums the pairs.
  3. Store yT [d, tok]; the host transposes shards back and concatenates.

FFN matmuls run in bf16 (fast weight load, 1 cyc/row); gating runs in exact
fp32. Weight tiles are DMA-staged fp32 then cast to bf16 on ACT/DVE.
"""

import numpy as np

import bass_rust
import concourse.bass as bass
import concourse.tile as tile
from concourse import mybir
from concourse.bass_utils import run_bass_kernel_spmd
from concourse.masks import make_identity
from concourse.tile_rust import add_dep_helper

N_CORES = 8
B, S, D, H, E = 4, 2048, 1024, 512, 8
NTOK = B * S          # 8192 total tokens
TOK = NTOK // N_CORES  # 1024 tokens per core
KD = D // 128          # 8 d_model chunks
KH = H // 128          # 4 hidden chunks
TT = TOK // 128        # 8 token chunks
NF = 512               # matmul moving free width
NHF = TOK // NF        # 2 token halves

FP = mybir.dt.float32
BF = mybir.dt.bfloat16
AF = mybir.ActivationFunctionType
ALU = mybir.AluOpType
AX = mybir.AxisListType


def _legalize_sync_waits(nc, max_waits=1):
    """Split multi-wait instructions for this walrus (1 sync wait per inst).

    Any instruction carrying more than ``max_waits`` sync-wait commands gets
    the extra waits peeled onto same-engine NoOps inserted immediately before
    it -- identical semantics (engine program order), legal ISA encoding.
    """
    n_split = 0
    for f in nc.m.functions:
        for bb in f.blocks:
            new_insts = []
            for inst in bb.instructions:
                si = getattr(inst, "sync_info", None)
                if si is not None and len(si.on_wait) > max_waits:
                    waits = list(si.on_wait)
                    for w in waits[max_waits:]:
                        nop = mybir.InstNoOp(
                            name=nc.get_next_instruction_name(), ins=[], outs=[]
                        )
                        nop.engine = inst.engine
                        nop.sync_info = bass_rust.SyncInfo(
                            on_wait=[w], on_update=[]
                        )
                        new_insts.append(nop)
                        n_split += 1
                    inst.sync_info = bass_rust.SyncInfo(
                        on_wait=waits[:max_waits], on_update=list(si.on_update)
                    )
                new_insts.append(inst)
            bb.instructions = new_insts
    return n_split


def _emit(tc, x, gw, w1, b1, w2, b2, outT):
    nc = tc.nc

    with (
        tc.tile_pool(name="const", bufs=1) as const_pool,
        tc.tile_pool(name="persist", bufs=1) as persist,
        tc.tile_pool(name="wstage", bufs=3) as wstage,
        tc.tile_pool(name="w1pool", bufs=3) as w1pool,
        tc.tile_pool(name="w2pool", bufs=3) as w2pool,
        tc.tile_pool(name="bpool", bufs=4) as bpool,
        tc.tile_pool(name="hpool", bufs=3) as hpool,
        tc.tile_pool(name="sbpool", bufs=3) as sbpool,
        tc.tile_pool(name="fpsum", bufs=3, space="PSUM") as fpsum,
    ):
        ident = const_pool.tile([128, 128], FP, tag="ident")
        make_identity(nc, ident[:])
        ones_f = const_pool.tile([1, 128], FP, tag="ones_f")
        nc.vector.memset(ones_f[:], 1.0)
        ones_row = const_pool.tile([1, 128], BF, tag="ones")
        nc.vector.tensor_copy(ones_row[:], ones_f[:])

        # gate_w [D, E] -> per-d-chunk [128, E] blocks, free-concatenated.
        # These are 32B-per-partition strided loads (1024 tiny descriptors);
        # they are emitted AFTER the x chunks so they never head-of-line
        # block the x stream, and gating only needs them ~40us in.
        gw_sb = const_pool.tile([128, KD * E], FP, tag="gw")
        # b2 [E, D] natural layout (E on partitions), cast to bf16
        b2f = const_pool.tile([E, D], FP, tag="b2f")
        nc.gpsimd.dma_start(b2f[:], b2[:, :])
        b2T = persist.tile([E, D], BF, tag="b2T")
        nc.vector.tensor_copy(b2T[:], b2f[:])
        # b1 [E, H] loaded whole (4 descriptors) and PE-transposed into the
        # per-partition bias layout b1t_all[:, mh*E+e] -- replaces 8 per-
        # expert strided loads of 512 x 4B descriptors each.
        b1nat = const_pool.tile([E, H], FP, tag="b1nat")
        nc.gpsimd.dma_start(b1nat[:], b1[:, :])
        b1t_all = persist.tile([128, KH * E], FP, tag="b1t_all")

        # bf16 xT for FFN matmuls; exact fp32 xg (stage-scoped) for gating so
        # the top-2 selection matches the oracle.
        xT = [
            persist.tile([128, TOK], BF, tag=f"xT{d}", name=f"xT{d}")
            for d in range(KD)
        ]
        ST = persist.tile([E, TOK], BF, tag="ST")
        acc = [
            persist.tile([128, TOK], BF, tag=f"acc{m}", name=f"acc{m}")
            for m in range(KD)
        ]

        # weight streaming: DMA on gpsimd (keeps the sync queue free for x),
        # bf16 casts on ACT; prefetched two experts ahead.
        loaded = {}

        def _stage_dma(dram_rows, n_chunks, chunk_free, after=None):
            stgs = []
            for k in range(n_chunks):
                stg = wstage.tile([128, chunk_free], FP, tag="ws", name="stg")
                di = nc.gpsimd.dma_start(
                    stg[:], dram_rows[k * 128:(k + 1) * 128, :]
                )
                if after is not None:
                    # hold this transfer until the prologue x loads finish
                    # so x keeps full HBM bandwidth
                    add_dep_helper(di.ins, after, reason="hbm x-priority")
                stgs.append(stg)
            return stgs

        def _cast_bf16(stgs, chunk_free, dst, use_act):
            # emitted late so these casts never head-of-line-block the
            # current expert's gelu (ACT) / scale (DVE) work
            for k, stg in enumerate(stgs):
                dsl = dst[:, k * chunk_free:(k + 1) * chunk_free]
                if use_act:
                    nc.scalar.copy(dsl, stg[:])
                else:
                    nc.vector.tensor_copy(dsl, stg[:])

        def _load_w1(e, after=None):
            w1t = w1pool.tile([128, KD * H], BF, tag="w1", name="w1t")
            _cast_bf16(
                _stage_dma(w1[e], KD, H, after=after), H, w1t, use_act=True
            )
            loaded[e] = (w1t, e)

        def _load_w2(e):
            w2t = w2pool.tile([128, KH * D], BF, tag="w2", name="w2t")
            _cast_bf16(_stage_dma(w2[e], KH, D), D, w2t, use_act=False)
            loaded_w2[e] = w2t

        def _l1_both(w1t, b1e, hts, sbt):
            # both token halves per (mh, kd) so each stationary tile is used
            # by two back-to-back matmuls (reload hidden / elided)
            for mh in range(KH):
                ph = [
                    fpsum.tile([128, NF], FP, tag="ph", name="ph")
                    for _ in range(NHF)
                ]
                for kd in range(KD):
                    stat = w1t[:, kd * H + mh * 128:kd * H + (mh + 1) * 128]
                    for hf in range(NHF):
                        nc.tensor.matmul(
                            ph[hf][:],
                            stat,
                            xT[kd][:, hf * NF:(hf + 1) * NF],
                            start=(kd == 0),
                            stop=(kd == KD - 1),
                        )
                for hf in range(NHF):
                    hsl = hts[:, mh * TOK + hf * NF:mh * TOK + (hf + 1) * NF]
                    nc.scalar.activation(
                        hsl, ph[hf][:], AF.Gelu,
                        bias=b1t_all[:, mh * E + b1e:mh * E + b1e + 1],
                    )
                    nc.vector.tensor_tensor(
                        hsl, hsl, sbt[:, hf * NF:(hf + 1) * NF], op=ALU.mult
                    )

        def _l1_half(w1t, b1e, hts, hf, sbt=None):
            # layer 1 for one token half: hts[:, mh, hf] = gelu(w1.T @ xT + b1)
            # scaled by the expert's per-token gate weight when sbt is given
            for mh in range(KH):
                ph = fpsum.tile([128, NF], FP, tag="ph", name="ph")
                for kd in range(KD):
                    nc.tensor.matmul(
                        ph[:],
                        w1t[:, kd * H + mh * 128:kd * H + (mh + 1) * 128],
                        xT[kd][:, hf * NF:(hf + 1) * NF],
                        start=(kd == 0),
                        stop=(kd == KD - 1),
                    )
                hsl = hts[:, mh * TOK + hf * NF:mh * TOK + (hf + 1) * NF]
                nc.scalar.activation(
                    hsl, ph[:], AF.Gelu,
                    bias=b1t_all[:, mh * E + b1e:mh * E + b1e + 1],
                )
                if sbt is not None:
                    nc.vector.tensor_tensor(
                        hsl, hsl, sbt[:, hf * NF:(hf + 1) * NF], op=ALU.mult
                    )

        loaded_w2 = {}
        xlast = {}
        hts_pair = {}
        w2_pair = {}

        # ---- stage 1: x load + transpose + gating, with expert-0 layer 1
        # interleaved so the PE stream stays dense (HAM warm) ---------------
        with (
            tc.tile_pool(name="xin", bufs=4) as xin_pool,
            tc.tile_pool(name="xg", bufs=1) as xg_pool,
            tc.tile_pool(name="tpsum", bufs=2, space="PSUM") as tpsum,
            tc.tile_pool(name="gpsum", bufs=1, space="PSUM") as gpsum,
            tc.tile_pool(name="gtmp", bufs=3) as gtmp,
        ):
            xg = [
                xg_pool.tile([128, TOK], FP, tag=f"xg{d}", name=f"xg{d}")
                for d in range(KD)
            ]

            def _tchunk(t):
                xt = xin_pool.tile([128, D], FP, tag="xt", name="xt")
                # split the 512KB tile load across 8 DMA queues on all
                # three DMA-capable engines (one queue only ~50 GB/s)
                engs = [nc.sync, nc.scalar, nc.gpsimd]
                for q in range(8):
                    di = engs[q % 3].dma_start(
                        xt[:, q * (D // 8):(q + 1) * (D // 8)],
                        x[t * 128:(t + 1) * 128,
                          q * (D // 8):(q + 1) * (D // 8)],
                    )
                    xlast[t] = di.ins
                for d in range(KD):
                    pt = tpsum.tile([128, 128], FP, tag="pt", name="pt")
                    nc.tensor.transpose(
                        pt[:], xt[:, d * 128:(d + 1) * 128], ident[:]
                    )
                    nc.vector.tensor_copy(
                        xg[d][:, t * 128:(t + 1) * 128], pt[:]
                    )
                    nc.vector.tensor_copy(
                        xT[d][:, t * 128:(t + 1) * 128], pt[:]
                    )
                # gating for this token chunk (exact fp32)
                ts = slice(t * 128, (t + 1) * 128)
                pg = gpsum.tile([128, E], FP, tag="pg", name="pg")
                for d in range(KD):
                    nc.tensor.matmul(
                        pg[:],
                        xg[d][:, ts],
                        gw_sb[:, d * E:(d + 1) * E],
                        start=(d == 0),
                        stop=(d == KD - 1),
                    )
                m = gtmp.tile([128, 1], FP, tag="m", name="m")
                nc.vector.tensor_reduce(m[:], pg[:], axis=AX.X, op=ALU.max)
                nm = gtmp.tile([128, 1], FP, tag="nm", name="nm")
                nc.vector.tensor_scalar(nm[:], m[:], -1.0, None, op0=ALU.mult)
                ex = gtmp.tile([128, E], FP, tag="ex", name="ex")
                nc.scalar.activation(ex[:], pg[:], AF.Exp, bias=nm[:, 0:1])
                ssum = gtmp.tile([128, 1], FP, tag="ssum", name="ssum")
                nc.vector.tensor_reduce(ssum[:], ex[:], axis=AX.X, op=ALU.add)
                r = gtmp.tile([128, 1], FP, tag="r", name="r")
                nc.vector.reciprocal(r[:], ssum[:])
                g = gtmp.tile([128, E], FP, tag="g", name="g")
                nc.vector.tensor_scalar(g[:], ex[:], r[:, 0:1], None, op0=ALU.mult)
                # top-2: m1 = max, m2 = max after suppressing the argmax
                m1 = gtmp.tile([128, 1], FP, tag="m1", name="m1")
                nc.vector.tensor_reduce(m1[:], g[:], axis=AX.X, op=ALU.max)
                is1 = gtmp.tile([128, E], FP, tag="is1", name="is1")
                nc.vector.tensor_scalar(
                    is1[:], g[:], m1[:, 0:1], None, op0=ALU.is_ge
                )
                g2 = gtmp.tile([128, E], FP, tag="g2", name="g2")
                nc.vector.tensor_scalar(g2[:], is1[:], -2.0, None, op0=ALU.mult)
                nc.vector.tensor_tensor(g2[:], g2[:], g[:], op=ALU.add)
                m2 = gtmp.tile([128, 1], FP, tag="m2", name="m2")
                nc.vector.tensor_reduce(m2[:], g2[:], axis=AX.X, op=ALU.max)
                tokw = gtmp.tile([128, 1], FP, tag="tokw", name="tokw")
                nc.vector.tensor_tensor(tokw[:], m1[:], m2[:], op=ALU.add)
                sel = gtmp.tile([128, E], FP, tag="sel", name="sel")
                nc.vector.tensor_scalar(
                    sel[:], g[:], m2[:, 0:1], None, op0=ALU.is_ge
                )
                sw = gtmp.tile([128, E], FP, tag="sw", name="sw")
                nc.vector.tensor_scalar(
                    sw[:], sel[:], tokw[:, 0:1], None, op0=ALU.mult
                )
                # transpose S chunk [128, E] -> ST[:, t*128:+128] (bf16)
                pst = gpsum.tile([128, 128], FP, tag="pst", name="pst")
                nc.tensor.transpose(pst[0:E, :], sw[:], ident[:])
                nc.vector.tensor_copy(ST[:, ts], pst[0:E, :])

            for mh in range(KH):
                ptb = tpsum.tile([128, 128], FP, tag="pt", name="ptb")
                nc.tensor.transpose(
                    ptb[:, 0:E], b1nat[:, mh * 128:(mh + 1) * 128],
                    ident[0:E, 0:E],
                )
                nc.vector.tensor_copy(
                    b1t_all[:, mh * E:(mh + 1) * E], ptb[:, 0:E]
                )
            for k in range(KD):
                eng = nc.sync if k % 2 == 0 else nc.scalar
                eng.dma_start(
                    gw_sb[:, k * E:(k + 1) * E], gw[k * 128:(k + 1) * 128, :]
                )
            # six chunks of transposes+gating give the PE dense work while
            # x streams at full HBM bandwidth; w1[0] transfers only start
            # once the first-half x chunks are in (dep edge), so expert 0's
            # layer 1 lands just-in-time after chunk 5
            for t in range(6):
                _tchunk(t)
            _load_w1(0, after=xlast[3])
            hts0 = hpool.tile([128, KH * TOK], BF, tag="h", name="hts0")
            hts_pair[0] = hts0
            _l1_half(loaded[0][0], loaded[0][1], hts0, 0)
            _tchunk(6)
            _tchunk(7)
            _load_w1(1, after=xlast[5])
            _l1_half(loaded[0][0], loaded[0][1], hts0, 1)

        # ---- stage 2: per-expert FFN (bf16), expert-pair PSUM accum -------
        with (
            tc.tile_pool(name="bpsum", bufs=1, space="PSUM") as bpsum,
            tc.tile_pool(name="ypsum", bufs=4, space="PSUM") as ypsum,
        ):
            def _sbt_for(e):
                # expert's S row to partition 0, then broadcast to all 128
                # partitions via a K=1 ones-matmul
                ste = sbpool.tile([1, TOK], BF, tag="ste", name="ste")
                nc.sync.dma_start(ste[:], ST[e:e + 1, :])
                sbt = sbpool.tile([128, TOK], BF, tag="sb", name="sbt")
                for hf in range(NHF):
                    pb = bpsum.tile([128, NF], FP, tag="pb", name="pb")
                    nc.tensor.matmul(
                        pb[:],
                        ones_row[:],
                        ste[:, hf * NF:(hf + 1) * NF],
                        start=True,
                        stop=True,
                    )
                    nc.vector.tensor_copy(sbt[:, hf * NF:(hf + 1) * NF], pb[:])
                return sbt

            def _scale(hts, sbt):
                for mh in range(KH):
                    for hf in range(NHF):
                        hsl = hts[
                            :, mh * TOK + hf * NF:mh * TOK + (hf + 1) * NF
                        ]
                        nc.vector.tensor_tensor(
                            hsl, hsl, sbt[:, hf * NF:(hf + 1) * NF],
                            op=ALU.mult,
                        )

            for e in range(E):
                eo = e % 2
                if e + 2 < E:
                    _load_w1(e + 2)
                w1t, b1t = loaded.pop(e)
                sbt = _sbt_for(e)
                if e > 0:
                    hts = hpool.tile([128, KH * TOK], BF, tag="h", name="hts")
                    hts_pair[eo] = hts
                    _l1_both(w1t, b1t, hts, sbt)
                else:
                    _scale(hts_pair[eo], sbt)
                _load_w2(e)
                w2_pair[eo] = loaded_w2.pop(e)
                if eo == 0:
                    continue
                # layer 2 for the expert pair (e-1, e), PSUM-accumulated
                for md in range(KD):
                    py = [
                        ypsum.tile([128, NF], FP, tag="py", name="py")
                        for _ in range(NHF)
                    ]
                    for po in (0, 1):
                        for kh in range(KH):
                            stat = w2_pair[po][
                                :, kh * D + md * 128:kh * D + (md + 1) * 128
                            ]
                            for hf in range(NHF):
                                nc.tensor.matmul(
                                    py[hf][:],
                                    stat,
                                    hts_pair[po][
                                        :,
                                        kh * TOK + hf * NF:kh * TOK + (hf + 1) * NF,
                                    ],
                                    start=(po == 0 and kh == 0),
                                    stop=(po == 1 and kh == KH - 1 and e != 1),
                                )
                    if e == 1:
                        # + sum_e S_e[tok] * b2[e, d] as a rank-8 matmul
                        for hf in range(NHF):
                            nc.tensor.matmul(
                                py[hf][:],
                                b2T[:, md * 128:(md + 1) * 128],
                                ST[:, hf * NF:(hf + 1) * NF],
                                start=False,
                                stop=True,
                            )
                    for hf in range(NHF):
                        asl = acc[md][:, hf * NF:(hf + 1) * NF]
                        if e == 1:
                            nc.vector.tensor_copy(asl, py[hf][:])
                        else:
                            nc.vector.tensor_tensor(
                                asl, asl, py[hf][:], op=ALU.add
                            )
                        if e == E - 1 and hf == NHF - 1:
                            # final pair: stream this d-chunk out right away,
                            # split across both HWDGE engines
                            eng = nc.sync if md % 2 == 0 else nc.scalar
                            eng.dma_start(
                                outT[md * 128:(md + 1) * 128, :TOK // 2],
                                acc[md][:, :TOK // 2],
                            )
                            eng2 = nc.scalar if md % 2 == 0 else nc.sync
                            eng2.dma_start(
                                outT[md * 128:(md + 1) * 128, TOK // 2:],
                                acc[md][:, TOK // 2:],
                            )


_CACHED_NC = None


def _build():
    global _CACHED_NC
    if _CACHED_NC is not None:
        return _CACHED_NC
    nc = bass.Bass(
        "TRN2", target_bir_lowering=False, debug=False, num_devices=N_CORES
    )
    x = nc.dram_tensor("x", [TOK, D], FP, kind="ExternalInput").ap()
    gw = nc.dram_tensor("gate_w", [D, E], FP, kind="ExternalInput").ap()
    w1 = nc.dram_tensor("w1", [E, D, H], FP, kind="ExternalInput").ap()
    b1 = nc.dram_tensor("b1", [E, H], FP, kind="ExternalInput").ap()
    w2 = nc.dram_tensor("w2", [E, H, D], FP, kind="ExternalInput").ap()
    b2 = nc.dram_tensor("b2", [E, D], FP, kind="ExternalInput").ap()
    outT = nc.dram_tensor("outT", [D, TOK], BF, kind="ExternalOutput").ap()
    with tile.TileContext(nc) as tc:
        _emit(tc, x, gw, w1, b1, w2, b2, outT)
    _legalize_sync_waits(nc)
    _CACHED_NC = nc
    return nc


def run(inputs, **spmd_kwargs):
    """Shard, run on 8 cores, unshard. Returns (out [B,S,D], BassKernelResults)."""
    nc = _build()
    xf = np.ascontiguousarray(
        np.asarray(inputs["x"], dtype=np.float32).reshape(NTOK, D)
    )
    shared = {
        k: np.ascontiguousarray(np.asarray(inputs[k], dtype=np.float32))
        for k in ("gate_w", "w1", "b1", "w2", "b2")
    }
    in_maps = [
        {"x": xf[c * TOK:(c + 1) * TOK], **shared} for c in range(N_CORES)
    ]
    res = run_bass_kernel_spmd(nc, in_maps, list(range(N_CORES)), **spmd_kwargs)
    out = np.concatenate(
        [np.asarray(res.results[c]["outT"]).astype(np.float32).T
         for c in range(N_CORES)], axis=0
    )
    return out.reshape(B, S, D), res


def kernel(**inputs):
    out, _ = run(inputs)
    return out

